# revision 32
# baseline (speedup 1.0000x reference)
"""MoE MLP (dense all-experts routing) Trainium2 Bass kernel.

Math (reference):
    g   = softmax(x @ gate_w + gate_b)            # [N, E]
    h   = relu(einsum("nd,edh->neh", x, w1) + b1) # [N, E, H]
    out = einsum("neh,ehd,ne->nd", h, w2, g)      # [N, D]

With E=64, H=16 (E*H = 1024 = D) this is two dense [1024,1024] matmuls plus a
small gate matmul.  Expert-hidden axis is reordered as eh' = h*64 + e
(h-major) so the gating multiply is a plain elementwise multiply of every
128-row tile of h^T by one shared [128, tok] tile of duplicated gate probs.

Layouts on device are feature-major (x^T, h^T, out^T); tokens are the matmul
moving (free) dimension.  Sharding: data-parallel over tokens, 4096 per core,
8 cores, no collectives.  Matmuls run in bf16 with fp32 PSUM accumulation.

Perf notes (vs the 263.7us baseline):
  - All DRAM->SBUF transfers are plain contiguous copies of host-prepped
    layouts (>=2KB per partition line); the baseline's rearranged 256B/1KB
    descriptors made the first x tile land at ~19us.  Consts ride the scalar
    HWDGE ring so they don't delay x0 on the sync ring.
  - Gate matmuls use 64-wide outputs col-tiled into both array halves
    (even k -> partitions 0:64, odd k -> 64:128, concurrent pairs), then one
    merge matmul against a 0/1 dup matrix adds the halves and duplicates to
    128 partitions for the eh'-ordered gating multiply: ~6 PE slots per tile
    instead of 9.
  - Warmup matmuls sized to cover the initial DMA wait keep the PE HAM
    un-throttled so real work starts warm.
"""

import numpy as np
import ml_dtypes

N, D, E, H = 32768, 1024, 64, 16
EH = E * H  # 1024
NCORES = 8
NTOK = N // NCORES  # tokens per core
TT = 512            # token tile (one PSUM bank of fp32)
KC = D // 128       # 8 contraction chunks for D
MC = EH // 128      # 8 output row-tiles for EH (and for D in stage 2)
WARMUP = 10         # N=512 warmup matmuls covering the initial DMA wait

_CACHE = {}


def build(n_tok=NTOK):
    """Build + compile the per-core Bass kernel for n_tok tokens."""
    import concourse.bass as bass
    import concourse.mybir as mybir
    import concourse.tile as tile
    from concourse import bacc

    f32 = mybir.dt.float32
    bf16 = mybir.dt.bfloat16
    AF = mybir.ActivationFunctionType
    nt = n_tok // TT
    assert n_tok % TT == 0

    nc = bacc.Bacc("TRN2", target_bir_lowering=False, debug=False)

    xd = nc.dram_tensor("xd", [nt, 2, 128, KC // 2, TT], bf16, kind="ExternalInput")
    # all small consts packed into one tensor = one DMA (per-partition
    # descriptors cost ~0.26us/engine regardless of size, so four separate
    # small DMAs would burn ~8us of ring time):
    # cols [0:512] gate_w (k-major), [512:640] dup matrix, [640] gate_b
    # (duplicated), [641:649] b1 m-columns -- gb/b1 biases in bf16
    miscd = nc.dram_tensor("miscd", [128, 656], bf16, kind="ExternalInput")
    # weights in m-block PAIRS so each DMA is 4KB/partition (4KB descriptors
    # get full ring throughput; 2KB ones don't)
    w1d = nc.dram_tensor("w1d", [MC // 2, 128, 2, KC, 128], bf16, kind="ExternalInput")
    w2d = nc.dram_tensor("w2d", [MC // 2, 128, 2, KC, 128], bf16, kind="ExternalInput")
    outT = nc.dram_tensor("outT", [nt, MC, 128, TT], bf16, kind="ExternalOutput")

    with tile.TileContext(nc) as tc:
        with (
            tc.tile_pool(name="consts", bufs=1) as consts,
            tc.tile_pool(name="xp", bufs=1) as xp,
            tc.tile_pool(name="sp", bufs=2) as sp,
            tc.tile_pool(name="hp", bufs=3) as hp,
            tc.tile_pool(name="ps1", bufs=1, space=bass.MemorySpace.PSUM) as ps1,
            tc.tile_pool(name="ps2", bufs=3, space=bass.MemorySpace.PSUM) as ps2,
            tc.tile_pool(name="ps3", bufs=3, space=bass.MemorySpace.PSUM) as ps3,
        ):
            # --- delivery plan: ring FIFO is the priority mechanism.
            # sync ring: xa0 -> w1 pairs -> xa tiles/odd outputs.
            # scalar ring: misc -> xb0 -> w2 pairs -> xb tiles.
            # gpsimd ring: even outputs only.
            # x pool is single-buffered so tile t+1's x DMA can't hoist
            # ahead of the weight stream (it waits for tile t's buffer). ---
            # halfZ of 0.5 contracts both duplicated e2 halves over all 128
            # partitions (= sum over the 64 experts); full-row LDWEIGHTS
            # keeps pull-ahead pipelining (a 64-row Z would block it)
            halfZ = consts.tile([128, 128], bf16)
            nc.gpsimd.memset(halfZ[:], 0.5)
            misc = consts.tile([128, 656], bf16)
            nc.sync.dma_start(out=misc[:], in_=miscd[:])

            xa0 = xp.tile([128, KC // 2, TT], bf16, tag="xa")
            nc.sync.dma_start(out=xa0[:], in_=xd[0, 0])
            xb0 = xp.tile([128, KC // 2, TT], bf16, tag="xb")
            nc.scalar.dma_start(out=xb0[:], in_=xd[0, 1])

            def gw_k(k):
                return misc[:, k * 64:(k + 1) * 64]

            dup_sb = misc[:, 512:640]
            gb_sb = misc[:, 640:641]

            def b1_m(m):
                return misc[:, 641 + m:642 + m]

            # w1 pairs alternate across both HWDGE rings (need-ordered);
            # w2 pairs queue behind w1 on the scalar ring
            w1_sb = consts.tile([128, MC, KC, 128], bf16)
            for p in range(MC // 2):
                eng = nc.sync if p % 2 == 0 else nc.scalar
                eng.dma_start(out=w1_sb[:, 2 * p:2 * p + 2], in_=w1d[p])
            w2_sb = consts.tile([128, MC, KC, 128], bf16)
            for p in range(MC // 2):
                nc.scalar.dma_start(out=w2_sb[:, 2 * p:2 * p + 2], in_=w2d[p])

            # HAM warmup: dummy matmuls on a zeroed scratch tile fill the
            # initial weight/x DMA wait and un-throttle the PE clock gate
            # (4/8 -> 8/8) before real work arrives.  Uses the lg PSUM slot,
            # which the first real gate group then reuses.
            wsc = consts.tile([128, TT], bf16, tag="wsc")
            nc.vector.memset(wsc[:], 0.0)
            # warmups use the zb slot (first real use: the merge, ~2.5us
            # after the gate) so the gate's lg group has no dependency on
            # them and the scheduler keeps it first in the PE queue
            wps = ps1.tile([128, TT], f32, tag="zb")
            for i in range(WARMUP):
                nc.tensor.matmul(wps[:], wsc[:, 0:128], wsc[:],
                                 start=(i == 0), stop=(i == WARMUP - 1))

            for t in range(nt):
                # x tile in two k-halves on separate HWDGE rings so each
                # tile lands as early as possible
                if t == 0:
                    xa, xb = xa0, xb0
                else:
                    xa = xp.tile([128, KC // 2, TT], bf16, tag="xa")
                    nc.sync.dma_start(out=xa[:], in_=xd[t, 0])
                    xb = xp.tile([128, KC // 2, TT], bf16, tag="xb")
                    nc.scalar.dma_start(out=xb[:], in_=xd[t, 1])

                def xk(k):
                    return (xa if k < KC // 2 else xb)[:, k % (KC // 2), :]

                # --- gate logits, col-tiled: even k chunks accumulate into
                # partitions 0:64 (array cols 0-63), odd into 64:128 --
                # consecutive pairs run concurrently in the array halves.
                lg = ps1.tile([128, TT], f32, tag="lg")
                for j in range(4):
                    nc.tensor.matmul(
                        lg[0:64, :], gw_k(2 * j), xk(2 * j),
                        start=(j == 0), stop=(j == 3), skip_group_check=True,
                    )
                    nc.tensor.matmul(
                        lg[64:128, :], gw_k(2 * j + 1), xk(2 * j + 1),
                        start=(j == 0), stop=(j == 3), skip_group_check=True,
                    )
                lgsb = sp.tile([128, TT], bf16, tag="lgsb")
                nc.vector.tensor_copy(lgsb[:], lg[:])

                hg = sp.tile([128, MC, TT], bf16, tag="hg")
                h_tiles = []

                def stage1_mm(m):
                    hps = ps2.tile([128, TT], f32, tag="hps")
                    for k in range(KC):
                        nc.tensor.matmul(
                            hps[:], w1_sb[:, m, k, :], xk(k),
                            start=(k == 0), stop=(k == KC - 1),
                        )
                    return hps

                def stage1_act(m, hps):
                    h = hp.tile([128, TT], bf16, tag="h")
                    nc.scalar.activation(
                        h[:], hps[:], AF.Relu, bias=b1_m(m), scale=1.0
                    )
                    h_tiles.append((m, h))

                def stage1(m):
                    stage1_act(m, stage1_mm(m))

                # interleave stage-1 blocks into the gate chain so the PE
                # never waits on DVE/ACT latency (merge needs the lgsb copy,
                # Z needs the exp); exp is emitted before relu(0) so it runs
                # first in the scalar queue
                hps0 = stage1_mm(0)

                # merged+duplicated logits: out[m] = lg[m%64] + lg[64+m%64]
                e2l = ps1.tile([128, TT], f32, tag="zb")
                nc.tensor.matmul(e2l[:], dup_sb[:], lgsb[:], start=True, stop=True)
                e2 = sp.tile([128, TT], bf16, tag="e2")
                nc.scalar.activation(e2[:], e2l[:], AF.Exp, bias=gb_sb[:], scale=1.0)
                stage1_act(0, hps0)

                stage1(1)

                # Z = sum of exp over the 64 experts, broadcast to all 128
                # partitions (0.5 * both duplicated halves), then g2 = e2/Z
                zb = ps1.tile([128, TT], f32, tag="zb")
                nc.tensor.matmul(zb[:], halfZ[:], e2[:], start=True, stop=True)

                stage1(2)

                rzb = sp.tile([128, TT], f32, tag="rzb")
                nc.vector.reciprocal_approx_fast(rzb[:], zb[:])
                g2 = sp.tile([128, TT], bf16, tag="g2")
                nc.vector.tensor_mul(g2[:], e2[:], rzb[:])

                for m, h in h_tiles:
                    nc.vector.tensor_mul(hg[:, m, :], h[:], g2[:])
                for m in range(3, MC):
                    stage1(m)
                    _, h = h_tiles[-1]
                    nc.vector.tensor_mul(hg[:, m, :], h[:], g2[:])

                # --- stage 2: out^T tiles ---
                last = t == nt - 1
                for m2 in range(MC):
                    ops = ps3.tile([128, TT], f32, tag="ops")
                    for k in range(MC):
                        nc.tensor.matmul(
                            ops[:], w2_sb[:, m2, k, :], hg[:, k, :],
                            start=(k == 0), stop=(k == MC - 1),
                        )
                    osb = hp.tile([128, TT], bf16, tag="osb")
                    nc.vector.tensor_copy(osb[:], ops[:])
                    # alternate output rings: halves the gpsimd ring load
                    # and lets the last tile's DMAs drain in parallel
                    if m2 % 2 == 1:
                        nc.sync.dma_start(out=outT[t, m2], in_=osb[:])
                    else:
                        nc.gpsimd.dma_start(out=outT[t, m2], in_=osb[:])

    nc.compile()
    return nc


def host_prep(x, gate_w, gate_b, w1, b1, w2):
    bf = ml_dtypes.bfloat16
    f32 = np.float32
    nt = NTOK // TT
    xb = x.astype(bf)
    x_shards = [
        np.ascontiguousarray(
            xb[c * NTOK:(c + 1) * NTOK]
            .reshape(nt, TT, 2, KC // 2, 128).transpose(0, 2, 4, 3, 1)
        )
        for c in range(NCORES)
    ]
    miscd = np.zeros((128, 656), dtype=bf)
    miscd[:, 0:512] = (
        gate_w.astype(bf).reshape(KC, 128, 64).transpose(1, 0, 2)
        .reshape(128, 512))
    ki = np.arange(128)[:, None]
    mi = np.arange(128)[None, :]
    miscd[:, 512:640] = ((ki % 64) == (mi % 64)).astype(bf)
    miscd[:, 640] = np.concatenate([gate_b, gate_b]).astype(bf)
    # eh' = h*64 + e ordering
    miscd[:, 641:649] = b1.T.reshape(EH).astype(bf).reshape(MC, 128).T
    w1d = np.ascontiguousarray(
        w1.transpose(1, 2, 0).reshape(D, EH).astype(bf)
        .reshape(KC, 128, MC // 2, 2, 128).transpose(2, 1, 3, 0, 4))
    w2d = np.ascontiguousarray(
        w2.transpose(1, 0, 2).reshape(EH, D).astype(bf)
        .reshape(KC, 128, MC // 2, 2, 128).transpose(2, 1, 3, 0, 4))
    common = {"miscd": miscd, "w1d": w1d, "w2d": w2d}
    return x_shards, common


def kernel(x, gate_w, gate_b, w1, b1, w2, _trace=False):
    import concourse.bass_utils as bass_utils

    x = np.asarray(x, dtype=np.float32)
    gate_w = np.asarray(gate_w, dtype=np.float32)
    gate_b = np.asarray(gate_b, dtype=np.float32)
    w1 = np.asarray(w1, dtype=np.float32)
    b1 = np.asarray(b1, dtype=np.float32)
    w2 = np.asarray(w2, dtype=np.float32)

    if "nc" not in _CACHE:
        _CACHE["nc"] = build(NTOK)
    nc = _CACHE["nc"]

    x_shards, common = host_prep(x, gate_w, gate_b, w1, b1, w2)
    in_maps = [dict(common, xd=x_shards[c]) for c in range(NCORES)]
    try:
        res = bass_utils.run_bass_kernel_spmd(
            nc, in_maps, core_ids=list(range(NCORES)), trace=_trace
        )
    except Exception:
        # transient device states (e.g. NRT_EXEC_UNIT_UNRECOVERABLE after a
        # wedged prior run) usually clear after a pause; retry once
        import time
        time.sleep(30)
        res = bass_utils.run_bass_kernel_spmd(
            nc, in_maps, core_ids=list(range(NCORES)), trace=_trace
        )
    _CACHE["last_results"] = res
    nt = NTOK // TT
    outs = [
        r["outT"].reshape(nt, MC, 128, TT).transpose(0, 3, 1, 2).reshape(NTOK, D)
        for r in res.results
    ]
    return np.ascontiguousarray(np.concatenate(outs, axis=0), dtype=np.float32)
